# revision 1
# baseline (speedup 1.0000x reference)
"""AttentionSequencePoolingLayer Trainium2 kernel (8-core data parallel).

B=2048, S=200, D=64, H1=64, H2=16. Batch sharded 256/core.
Per core, per pair of batch rows (b0,b1):
  kT pair tiles [128=(bhat,d), 128/80 tok] via SWDGE cast-load + xbar transpose.
  z1 = blkdiag(Wk)^T kT + blkdiag(Wqk)^T (q*k)    (PSUM [128,208])
  p1 = sigmoid(s1*z1 + sb1)  [ACT, per-partition scale/bias]
  x1 = z1 + qW_b             [DVE TS]
  u  = x1*p1                 [DVE TT]
  z2 = blkdiag(a1*W2)^T x1 + blkdiag((1-a1)*W2)^T u   (dice1 folded into L2)
  h2 = z2 * (a2 + (1-a2)*sigmoid(s2*z2 + t2))          (batched 4 pairs)
  scores = h2-as-lhsT @ blkdiag(W3)  -> token-major PSUM
  w = sigmoid(scores) * mask
  out_b = w^T k  (PE, K=tokens)
"""
import numpy as np
import ml_dtypes

import concourse.bacc as bacc
import concourse.tile as tile
import concourse.mybir as mybir
import concourse.bass as bass
from concourse.bass_utils import run_bass_kernel_spmd

B, S, D = 2048, 200, 64
H1, H2 = 64, 16
EPS = 1e-9
NCORES = 8
BLOC = B // NCORES          # 256 batch rows per core
NGROUPS_FULL = BLOC // 16   # 16

F32 = mybir.dt.float32
BF16 = mybir.dt.bfloat16
AF = mybir.ActivationFunctionType
ALU = mybir.AluOpType
bf = ml_dtypes.bfloat16

_CACHE = {}
TRACE = False
LAST_RESULT = None


def _build(ngroups):
    nc = bacc.Bacc("TRN2", target_bir_lowering=False, debug=False, num_devices=NCORES)
    nb = 16 * ngroups           # batch rows this build processes
    npair = nb // 2

    key = nc.dram_tensor("key", [nb * S, D], F32, kind="ExternalInput").ap()
    qp = nc.dram_tensor("qp", [128, npair], F32, kind="ExternalInput").ap()
    sb1p = nc.dram_tensor("sb1p", [128, npair], F32, kind="ExternalInput").ap()
    qwp = nc.dram_tensor("qwp", [128, npair], F32, kind="ExternalInput").ap()
    mask = nc.dram_tensor("mask", [128, 4 * npair], BF16, kind="ExternalInput").ap()
    wk2 = nc.dram_tensor("wk2", [128, 128], BF16, kind="ExternalInput").ap()
    wqk2 = nc.dram_tensor("wqk2", [128, 128], BF16, kind="ExternalInput").ap()
    w2a = nc.dram_tensor("w2a", [128, 32], BF16, kind="ExternalInput").ap()
    w2na = nc.dram_tensor("w2na", [128, 32], BF16, kind="ExternalInput").ap()
    w34 = nc.dram_tensor("w34", [128, 2], BF16, kind="ExternalInput").ap()
    cols = nc.dram_tensor("cols", [128, 6], F32, kind="ExternalInput").ap()
    # cols: 0=s1col 1=na1?? unused 2=s2col 3=b2col 4=na2col 5=a2col
    out = nc.dram_tensor("out", [nb, D], F32, kind="ExternalOutput").ap()

    key_r = key.rearrange("(b s) d -> s b d", s=S)  # [200, nb, 64] view

    with tile.TileContext(nc) as tc:
        with (
            tc.tile_pool(name="const", bufs=1) as cp,
            tc.tile_pool(name="load", bufs=2) as lp,
            tc.tile_pool(name="kt", bufs=2) as ktp,
            tc.tile_pool(name="work", bufs=3) as wp,
            tc.tile_pool(name="h2p", bufs=2) as h2p,
            tc.tile_pool(name="outp", bufs=2) as op_,
            tc.tile_pool(name="ps1", bufs=2, space="PSUM") as ps1,
            tc.tile_pool(name="ps2", bufs=2, space="PSUM") as ps2,
            tc.tile_pool(name="ps3", bufs=2, space="PSUM") as ps3,
            tc.tile_pool(name="ps4", bufs=2, space="PSUM") as ps4,
        ):
            # ---- constants into SBUF
            c_qp = cp.tile([128, npair], F32)
            nc.sync.dma_start(out=c_qp[:], in_=qp)
            c_sb1 = cp.tile([128, npair], F32)
            nc.sync.dma_start(out=c_sb1[:], in_=sb1p)
            c_qw = cp.tile([128, npair], F32)
            nc.sync.dma_start(out=c_qw[:], in_=qwp)
            c_mask = cp.tile([128, 4 * npair], BF16)
            nc.sync.dma_start(out=c_mask[:], in_=mask)
            c_wk = cp.tile([128, 128], BF16)
            nc.sync.dma_start(out=c_wk[:], in_=wk2)
            c_wqk = cp.tile([128, 128], BF16)
            nc.sync.dma_start(out=c_wqk[:], in_=wqk2)
            c_w2a = cp.tile([128, 32], BF16)
            nc.sync.dma_start(out=c_w2a[:], in_=w2a)
            c_w2na = cp.tile([128, 32], BF16)
            nc.sync.dma_start(out=c_w2na[:], in_=w2na)
            c_w34 = cp.tile([128, 2], BF16)
            nc.sync.dma_start(out=c_w34[:], in_=w34)
            c_cols = cp.tile([128, 6], F32)
            nc.sync.dma_start(out=c_cols[:], in_=cols)

            for g in range(ngroups):
                gf = lp.tile([128, 16, 64], BF16, tag="gf")
                nc.gpsimd.dma_start(out=gf[:], in_=key_r[0:128, 16 * g : 16 * g + 16, :])
                gp = lp.tile([128, 16, 64], BF16, tag="gp")
                prow = 72 if g == ngroups - 1 else 80
                if g >= ngroups - 2:
                    nc.vector.memset(gp[64:96, :, :], 0.0)
                nc.gpsimd.dma_start(
                    out=gp[0:prow, :, :],
                    in_=bass.AP(
                        key.tensor,
                        (16 * g * S + 128) * D,
                        [[D, prow], [S * D, 16], [1, D]],
                    ),
                )
                ktf = ktp.tile([128, 8, 128], BF16, tag="ktf")
                nc.sync.dma_start(out=ktf[:], in_=gf.rearrange("p b d -> p (b d)"), transpose=True)
                ktq = ktp.tile([128, 8, 80], BF16, tag="ktq")
                nc.sync.dma_start(
                    out=ktq[:],
                    in_=gp[0:80, :, :].rearrange("p b d -> p (b d)"),
                    transpose=True,
                )

                scores = ps3.tile([128, 32], F32, tag="sc")
                nc.vector.memset(scores[:], 0.0)
                z2 = None
                h2 = None
                for jw in range(8):
                    j = 8 * g + jw
                    kf = ktf[:, jw, :]
                    kq = ktq[:, jw, :]
                    qkf = wp.tile([128, 128], BF16, tag="qkf")
                    nc.vector.tensor_scalar(qkf[:], kf, c_qp[:, j : j + 1], None, ALU.mult)
                    qkq = wp.tile([128, 80], BF16, tag="qkq")
                    nc.vector.tensor_scalar(qkq[:], kq, c_qp[:, j : j + 1], None, ALU.mult)

                    z1 = ps1.tile([128, 208], F32, tag="z1")
                    nc.tensor.matmul(z1[:, 0:128], c_wk[:], kf, start=True, stop=False)
                    nc.tensor.matmul(z1[:, 0:128], c_wqk[:], qkf[:], start=False, stop=True)
                    nc.tensor.matmul(z1[:, 128:208], c_wk[:], kq, start=True, stop=False)
                    nc.tensor.matmul(z1[:, 128:208], c_wqk[:], qkq[:], start=False, stop=True)

                    p1 = wp.tile([128, 208], BF16, tag="p1")
                    nc.scalar.activation(p1[:], z1[:], AF.Sigmoid,
                                         bias=c_sb1[:, j : j + 1], scale=c_cols[:, 0:1])
                    x1 = wp.tile([128, 208], BF16, tag="x1")
                    nc.vector.tensor_scalar(x1[:], z1[:], c_qw[:, j : j + 1], None, ALU.add)
                    u1 = wp.tile([128, 208], BF16, tag="u1")
                    nc.vector.tensor_tensor(u1[:], x1[:], p1[:], ALU.mult)

                    jq = jw % 4
                    if jq == 0:
                        z2 = ps2.tile([128, 208], F32, tag="z2")
                    zb = 32 * jq
                    nc.tensor.matmul(z2[zb : zb + 32, :], c_w2a[:], x1[:],
                                     start=True, stop=False, tile_position=(0, zb))
                    nc.tensor.matmul(z2[zb : zb + 32, :], c_w2na[:], u1[:],
                                     start=False, stop=True, tile_position=(0, zb))

                    if jq == 3:
                        p2 = wp.tile([128, 208], BF16, tag="p2")
                        nc.scalar.activation(p2[:], z2[:], AF.Sigmoid,
                                             bias=c_cols[:, 3:4], scale=c_cols[:, 2:3])
                        t2 = wp.tile([128, 208], BF16, tag="t2")
                        nc.vector.tensor_scalar(t2[:], p2[:], c_cols[:, 4:5], c_cols[:, 5:6],
                                                ALU.mult, ALU.add)
                        h2 = h2p.tile([128, 208], BF16, tag="h2")
                        nc.vector.tensor_tensor(h2[:], z2[:], t2[:], ALU.mult)
                        for jj in range(4):
                            jb = 32 * jj
                            jw2 = jw - 3 + jj
                            nc.tensor.matmul(scores[0:128, 4 * jw2 : 4 * jw2 + 2],
                                             h2[jb : jb + 32, 0:128], c_w34[jb : jb + 32, :],
                                             start=True, stop=True, tile_position=(jb, 0))
                            nc.tensor.matmul(scores[0:80, 4 * jw2 + 2 : 4 * jw2 + 4],
                                             h2[jb : jb + 32, 128:208], c_w34[jb : jb + 32, :],
                                             start=True, stop=True, tile_position=(jb, 0))

                # sigmoid + mask for whole group
                sg = wp.tile([128, 32], BF16, tag="sg")
                nc.scalar.activation(sg[:], scores[:], AF.Sigmoid)
                wt = wp.tile([128, 32], BF16, tag="wt")
                nc.vector.tensor_tensor(wt[:], sg[:], c_mask[:, 32 * g : 32 * g + 32], ALU.mult)

                # pooling
                pool = ps4.tile([128, 256], F32, tag="pool")
                for jw in range(8):
                    pb = 32 * (jw // 2)
                    po = 128 * (jw % 2)
                    rhs_f = gf[:, 2 * jw : 2 * jw + 2, :].rearrange("p b d -> p (b d)")
                    rhs_p = gp[0:80, 2 * jw : 2 * jw + 2, :].rearrange("p b d -> p (b d)")
                    nc.tensor.matmul(pool[pb : pb + 2, po : po + 128],
                                     wt[0:128, 4 * jw : 4 * jw + 2], rhs_f,
                                     start=True, stop=False, tile_position=(0, pb))
                    nc.tensor.matmul(pool[pb : pb + 2, po : po + 128],
                                     wt[0:80, 4 * jw + 2 : 4 * jw + 4], rhs_p,
                                     start=False, stop=True, tile_position=(0, pb))

                po_sb = op_.tile([128, 256], F32, tag="po")
                nc.scalar.copy(po_sb[0:2, :], pool[0:2, :])
                nc.vector.tensor_copy(po_sb[32:34, :], pool[32:34, :])
                nc.scalar.copy(po_sb[64:66, :], pool[64:66, :])
                nc.vector.tensor_copy(po_sb[96:98, :], pool[96:98, :])

                # out rows: b = 16g + 4*bi + 2*jj + bhat ; sbuf row 32*bi+bhat, col 128*jj + 64*bhat + d
                for bh in range(2):
                    src = po_sb[bh:128:32, :].rearrange("p (jj x) -> p jj x", x=128)
                    src = src[:, :, 64 * bh : 64 * bh + 64]
                    dst = bass.AP(out.tensor, (16 * g + bh) * D,
                                  [[4 * D, 4], [2 * D, 2], [1, D]])
                    nc.sync.dma_start(out=dst, in_=src)
    nc.compile()
    return nc


def _prep_consts(W1, alpha1, mean1, var1, W2, alpha2, mean2, var2, W3):
    inv1 = 1.0 / np.sqrt(var1 + EPS)
    inv2 = 1.0 / np.sqrt(var2 + EPS)
    Wq = W1[0:64] + W1[128:192]
    Wk = W1[64:128] - W1[128:192]
    Wqk = W1[192:256]

    def blk(a):
        m = np.zeros((128, 2 * a.shape[1]), np.float32)
        m[0:64, 0 : a.shape[1]] = a
        m[64:128, a.shape[1] :] = a
        return m

    wk2 = blk(Wk).astype(bf)
    wqk2 = blk(Wqk).astype(bf)
    w2a = blk(np.diag(alpha1) @ W2).astype(bf)
    w2na = blk(np.diag(1.0 - alpha1) @ W2).astype(bf)
    w34p = np.zeros((32, 2), np.float32)
    w34p[0:16, 0] = W3[:, 0]
    w34p[16:32, 1] = W3[:, 0]
    w34 = np.tile(w34p, (4, 1)).astype(bf)
    cols = np.zeros((128, 6), np.float32)
    cols[:, 0] = np.tile(inv1, 2)
    cols[:, 2] = np.tile(inv2, 8)
    cols[:, 3] = np.tile(-mean2 * inv2, 8)
    cols[:, 4] = np.tile(1.0 - alpha2, 8)
    cols[:, 5] = np.tile(alpha2, 8)
    return Wq, mean1, inv1, wk2, wqk2, w2a, w2na, w34, cols


def kernel(query_emb, key_emb, seq_length, W1, alpha1, mean1, var1,
           W2, alpha2, mean2, var2, W3, _ngroups=NGROUPS_FULL):
    (Wq, m1, inv1, wk2, wqk2, w2a, w2na, w34, cols) = _prep_consts(
        np.asarray(W1, np.float32), np.asarray(alpha1, np.float32),
        np.asarray(mean1, np.float32), np.asarray(var1, np.float32),
        np.asarray(W2, np.float32), np.asarray(alpha2, np.float32),
        np.asarray(mean2, np.float32), np.asarray(var2, np.float32),
        np.asarray(W3, np.float32))
    q = np.asarray(query_emb, np.float32)
    k = np.asarray(key_emb, np.float32)
    sl = np.asarray(seq_length).reshape(-1)

    if _ngroups not in _CACHE:
        _CACHE[_ngroups] = _build(_ngroups)
    nc = _CACHE[_ngroups]
    nb = 16 * _ngroups
    npair = nb // 2

    qW = q @ Wq  # [B, 64]
    sb1_full = (qW - m1) * inv1

    in_maps = []
    for c in range(NCORES):
        b0 = c * BLOC
        qs = q[b0 : b0 + nb]
        qWs = qW[b0 : b0 + nb]
        sbs = sb1_full[b0 : b0 + nb]
        sls = sl[b0 : b0 + nb]
        # pair layouts [128=(bhat,d/h), npair]
        qp_t = np.zeros((128, npair), np.float32)
        sb1p_t = np.zeros((128, npair), np.float32)
        qwp_t = np.zeros((128, npair), np.float32)
        for bh in range(2):
            qp_t[64 * bh : 64 * bh + 64] = qs[bh::2].T
            sb1p_t[64 * bh : 64 * bh + 64] = sbs[bh::2].T
            qwp_t[64 * bh : 64 * bh + 64] = qWs[bh::2].T
        # mask [128, 4*npair]: cols 4j+c: c in {0,1}: full chunk b0/b1; {2,3}: part chunk
        t_full = np.arange(128)[:, None]
        t_part = np.arange(128)[:, None] + 128
        mk = np.zeros((128, 4 * npair), np.float32)
        mk[:, 0::4] = t_full < sls[0::2][None, :]
        mk[:, 1::4] = t_full < sls[1::2][None, :]
        mp0 = (t_part < sls[0::2][None, :]).astype(np.float32)
        mp1 = (t_part < sls[1::2][None, :]).astype(np.float32)
        mp0[80:] = 0.0
        mp1[80:] = 0.0
        mk[:, 2::4] = mp0
        mk[:, 3::4] = mp1
        in_maps.append({
            "key": k[b0 : b0 + nb].reshape(nb * S, D),
            "qp": qp_t.astype(np.float32), "sb1p": sb1p_t.astype(np.float32),
            "qwp": qwp_t.astype(np.float32), "mask": mk.astype(bf),
            "wk2": wk2, "wqk2": wqk2, "w2a": w2a, "w2na": w2na,
            "w34": w34, "cols": cols,
        })

    res = run_bass_kernel_spmd(nc, in_maps, list(range(NCORES)), trace=TRACE)
    global LAST_RESULT
    LAST_RESULT = res
    outs = []
    for c in range(NCORES):
        outs.append(res.results[c]["out"])
    return np.concatenate(outs, axis=0).astype(np.float32)



# revision 3
# speedup vs baseline: 1.0712x; 1.0712x over previous
"""AttentionSequencePoolingLayer Trainium2 kernel (8-core data parallel).

B=2048, S=200, D=64, H1=64, H2=16. Batch sharded 256/core.
Dataflow per core, per group of 16 batch rows (8 pairs, 2 tokens chunks
128+80):
  kT tiles via SWDGE cast-load + xbar transpose (as before).
  qk^T per pair via DVE tensor_scalar (2x mode).
  x1 = z1 + qW accumulated fully in PSUM: rank-2 one-hot matmul folds the
       per-pair qW broadcast, so dice-1 bias/scale become pair-independent.
  p1/t1/h1 batched over 2 pairs (416 cols) to amortize per-op overhead:
       p1 = sigmoid(inv1*x1 - m1*inv1)   [ACT]
       t1 = p1*(1-a1) + a1               [DVE TS 2x]
       h1 = x1 * t1                      [DVE TT, PSUM read]
  z2 = W2^T h1 — single matmul per pair (alpha folded into t1, not W2).
  p2/t2/h2 batched over all 8 pairs (416 cols).
  scores: block-diagonal W3 -> 4 matmuls per group (token-major PSUM).
  w = sigmoid(scores)*mask; out = w^T k on PE (as before).
"""
import numpy as np
import ml_dtypes

import concourse.bacc as bacc
import concourse.tile as tile
import concourse.mybir as mybir
import concourse.bass as bass
from concourse.bass_utils import run_bass_kernel_spmd

B, S, D = 2048, 200, 64
H1, H2 = 64, 16
EPS = 1e-9
NCORES = 8
BLOC = B // NCORES          # 256 batch rows per core
NGROUPS_FULL = BLOC // 16   # 16

F32 = mybir.dt.float32
BF16 = mybir.dt.bfloat16
AF = mybir.ActivationFunctionType
ALU = mybir.AluOpType
bf = ml_dtypes.bfloat16

_CACHE = {}
TRACE = False
LAST_RESULT = None


def _build(ngroups):
    nc = bacc.Bacc("TRN2", target_bir_lowering=False, debug=False, num_devices=NCORES)
    nb = 16 * ngroups           # batch rows this build processes
    npair = nb // 2

    key = nc.dram_tensor("key", [nb * S, D], F32, kind="ExternalInput").ap()
    qp = nc.dram_tensor("qp", [128, npair], F32, kind="ExternalInput").ap()
    qw2 = nc.dram_tensor("qw2", [2, 64 * npair], BF16, kind="ExternalInput").ap()
    onehot = nc.dram_tensor("onehot", [2, 416], BF16, kind="ExternalInput").ap()
    mask = nc.dram_tensor("mask", [128, 32 * ngroups], BF16, kind="ExternalInput").ap()
    wk2 = nc.dram_tensor("wk2", [128, 128], BF16, kind="ExternalInput").ap()
    wqk2 = nc.dram_tensor("wqk2", [128, 128], BF16, kind="ExternalInput").ap()
    w2b = nc.dram_tensor("w2b", [128, 32], BF16, kind="ExternalInput").ap()
    w34 = nc.dram_tensor("w34", [128, 8], BF16, kind="ExternalInput").ap()
    cols = nc.dram_tensor("cols", [128, 8], F32, kind="ExternalInput").ap()
    # cols: 0=inv1 1=-m1*inv1 2=1-a1 3=a1 4=inv2 5=-m2*inv2 6=1-a2 7=a2
    out = nc.dram_tensor("out", [nb, D], F32, kind="ExternalOutput").ap()

    key_r = key.rearrange("(b s) d -> s b d", s=S)  # [200, nb, 64] view

    with tile.TileContext(nc) as tc:
        with (
            tc.tile_pool(name="const", bufs=1) as cp,
            tc.tile_pool(name="load", bufs=2) as lp,
            tc.tile_pool(name="kt", bufs=2) as ktp,
            tc.tile_pool(name="qk", bufs=2) as qkp,
            tc.tile_pool(name="work", bufs=2) as wp,
            tc.tile_pool(name="h1p", bufs=3) as h1p,
            tc.tile_pool(name="outp", bufs=2) as op_,
            tc.tile_pool(name="psx", bufs=3, space="PSUM") as psx,
            tc.tile_pool(name="psz", bufs=2, space="PSUM") as psz,
            tc.tile_pool(name="psp", bufs=2, space="PSUM") as psp,
        ):
            # ---- constants into SBUF
            c_qp = cp.tile([128, npair], F32)
            nc.sync.dma_start(out=c_qp[:], in_=qp)
            c_qw2 = cp.tile([2, 64 * npair], BF16)
            nc.sync.dma_start(out=c_qw2[:], in_=qw2)
            c_oh = cp.tile([2, 416], BF16)
            nc.sync.dma_start(out=c_oh[:], in_=onehot)
            c_mask = cp.tile([128, 32 * ngroups], BF16)
            nc.sync.dma_start(out=c_mask[:], in_=mask)
            c_wk = cp.tile([128, 128], BF16)
            nc.sync.dma_start(out=c_wk[:], in_=wk2)
            c_wqk = cp.tile([128, 128], BF16)
            nc.sync.dma_start(out=c_wqk[:], in_=wqk2)
            c_w2 = cp.tile([128, 32], BF16)
            nc.sync.dma_start(out=c_w2[:], in_=w2b)
            c_w34 = cp.tile([128, 8], BF16)
            nc.sync.dma_start(out=c_w34[:], in_=w34)
            c_cols = cp.tile([128, 8], F32)
            nc.sync.dma_start(out=c_cols[:], in_=cols)

            for g in range(ngroups):
                gf = lp.tile([128, 16, 64], BF16, tag="gf")
                nc.gpsimd.dma_start(out=gf[:], in_=key_r[0:128, 16 * g : 16 * g + 16, :])
                gp = lp.tile([128, 16, 64], BF16, tag="gp")
                prow = 72 if g == ngroups - 1 else 80
                if g >= ngroups - 2:
                    nc.vector.memset(gp[64:96, :, :], 0.0)
                nc.gpsimd.dma_start(
                    out=gp[0:prow, :, :],
                    in_=bass.AP(
                        key.tensor,
                        (16 * g * S + 128) * D,
                        [[D, prow], [S * D, 16], [1, D]],
                    ),
                )
                ktf = ktp.tile([128, 8, 128], BF16, tag="ktf")
                nc.sync.dma_start(out=ktf[:], in_=gf.rearrange("p b d -> p (b d)"), transpose=True)
                ktq = ktp.tile([128, 8, 80], BF16, tag="ktq")
                nc.sync.dma_start(
                    out=ktq[:],
                    in_=gp[0:80, :, :].rearrange("p b d -> p (b d)"),
                    transpose=True,
                )

                # qk^T per pair (DVE 2x)
                qkf = qkp.tile([128, 8, 128], BF16, tag="qkf")
                qkq = qkp.tile([128, 8, 80], BF16, tag="qkq")
                for jj in range(8):
                    j = 8 * g + jj
                    nc.vector.tensor_scalar(qkf[:, jj, :], ktf[:, jj, :],
                                            c_qp[:, j : j + 1], None, ALU.mult)
                    nc.vector.tensor_scalar(qkq[:, jj, :], ktq[:, jj, :],
                                            c_qp[:, j : j + 1], None, ALU.mult)

                z2 = psz.tile([128, 416], F32, tag="z2")
                h1s = []
                for m in range(4):          # half-quads: pairs (2m, 2m+1)
                    x1 = psx.tile([128, 416], F32, tag="x1")
                    # rank-2 one-hot: x1[:, 208k:208k+208] = qW(pair 2m+k)
                    hq = 4 * g + m
                    nc.tensor.matmul(x1[:, 0:416],
                                     c_qw2[:, 128 * hq : 128 * hq + 128],
                                     c_oh[:], start=True, stop=False)
                    for k in range(2):
                        jj = 2 * m + k
                        base = 208 * k
                        nc.tensor.matmul(x1[:, base : base + 128], c_wk[:],
                                         ktf[:, jj, :], start=False, stop=False)
                        nc.tensor.matmul(x1[:, base : base + 128], c_wqk[:],
                                         qkf[:, jj, :], start=False, stop=True)
                        nc.tensor.matmul(x1[:, base + 128 : base + 208], c_wk[:],
                                         ktq[:, jj, :], start=False, stop=False)
                        nc.tensor.matmul(x1[:, base + 128 : base + 208], c_wqk[:],
                                         qkq[:, jj, :], start=False, stop=True)
                    p1 = wp.tile([128, 416], BF16, tag="p1")
                    nc.scalar.activation(p1[:], x1[:], AF.Sigmoid,
                                         bias=c_cols[:, 1:2], scale=c_cols[:, 0:1])
                    t1 = wp.tile([128, 416], BF16, tag="t1")
                    nc.vector.tensor_scalar(t1[:], p1[:], c_cols[:, 2:3], c_cols[:, 3:4],
                                            ALU.mult, ALU.add)
                    h1 = h1p.tile([128, 416], BF16, tag="h1")
                    nc.vector.tensor_tensor(h1[:], x1[:], t1[:], ALU.mult)
                    h1s.append(h1)
                    for k in range(2):
                        jj = 2 * m + k
                        q4, jq = jj // 4, jj % 4
                        nc.tensor.matmul(z2[32 * jq : 32 * jq + 32,
                                            208 * q4 : 208 * q4 + 208],
                                         c_w2[:], h1[:, 208 * k : 208 * k + 208],
                                         start=True, stop=True,
                                         tile_position=(0, 32 * jq))

                p2 = wp.tile([128, 416], BF16, tag="p2")
                nc.scalar.activation(p2[:], z2[:], AF.Sigmoid,
                                     bias=c_cols[:, 5:6], scale=c_cols[:, 4:5])
                t2 = wp.tile([128, 416], BF16, tag="t2")
                nc.vector.tensor_scalar(t2[:], p2[:], c_cols[:, 6:7], c_cols[:, 7:8],
                                        ALU.mult, ALU.add)
                h2 = wp.tile([128, 416], BF16, tag="h2")
                nc.vector.tensor_tensor(h2[:], z2[:], t2[:], ALU.mult)

                # scores: cols 0:16 = full chunk (8q+2jq+bh), 16:32 = partial
                scores = psz.tile([128, 32], F32, tag="sc", bufs=1)
                for q4 in range(2):
                    nc.tensor.matmul(scores[0:128, 8 * q4 : 8 * q4 + 8],
                                     h2[:, 208 * q4 : 208 * q4 + 128], c_w34[:],
                                     start=True, stop=True)
                    nc.tensor.matmul(scores[0:80, 16 + 8 * q4 : 24 + 8 * q4],
                                     h2[:, 208 * q4 + 128 : 208 * q4 + 208], c_w34[:],
                                     start=True, stop=True)

                sg = wp.tile([128, 32], BF16, tag="sg")
                nc.scalar.activation(sg[:, 0:16], scores[:, 0:16], AF.Sigmoid)
                nc.scalar.activation(sg[0:80, 16:32], scores[0:80, 16:32], AF.Sigmoid)
                wt = wp.tile([128, 32], BF16, tag="wt")
                nc.vector.tensor_tensor(wt[:, 0:16], sg[:, 0:16],
                                        c_mask[:, 32 * g : 32 * g + 16], ALU.mult)
                nc.vector.tensor_tensor(wt[0:80, 16:32], sg[0:80, 16:32],
                                        c_mask[0:80, 32 * g + 16 : 32 * g + 32], ALU.mult)

                # pooling
                pool = psp.tile([128, 256], F32, tag="pool")
                for jw in range(8):
                    pb = 32 * (jw // 2)
                    po = 128 * (jw % 2)
                    rhs_f = gf[:, 2 * jw : 2 * jw + 2, :].rearrange("p b d -> p (b d)")
                    rhs_p = gp[0:80, 2 * jw : 2 * jw + 2, :].rearrange("p b d -> p (b d)")
                    nc.tensor.matmul(pool[pb : pb + 2, po : po + 128],
                                     wt[0:128, 2 * jw : 2 * jw + 2], rhs_f,
                                     start=True, stop=False, tile_position=(0, pb))
                    nc.tensor.matmul(pool[pb : pb + 2, po : po + 128],
                                     wt[0:80, 16 + 2 * jw : 16 + 2 * jw + 2], rhs_p,
                                     start=False, stop=True, tile_position=(0, pb))

                po_sb = op_.tile([128, 256], F32, tag="po")
                nc.scalar.copy(po_sb[0:2, :], pool[0:2, :])
                nc.scalar.copy(po_sb[32:34, :], pool[32:34, :])
                nc.scalar.copy(po_sb[64:66, :], pool[64:66, :])
                nc.scalar.copy(po_sb[96:98, :], pool[96:98, :])

                # out rows: b = 16g + 4*bi + 2*jj + bhat ; sbuf row 32*bi+bhat, col 128*jj + 64*bhat + d
                for bh in range(2):
                    src = po_sb[bh:128:32, :].rearrange("p (jj x) -> p jj x", x=128)
                    src = src[:, :, 64 * bh : 64 * bh + 64]
                    dst = bass.AP(out.tensor, (16 * g + bh) * D,
                                  [[4 * D, 4], [2 * D, 2], [1, D]])
                    nc.sync.dma_start(out=dst, in_=src)
    nc.compile()
    return nc


def _prep_consts(W1, alpha1, mean1, var1, W2, alpha2, mean2, var2, W3):
    inv1 = 1.0 / np.sqrt(var1 + EPS)
    inv2 = 1.0 / np.sqrt(var2 + EPS)
    Wq = W1[0:64] + W1[128:192]
    Wk = W1[64:128] - W1[128:192]
    Wqk = W1[192:256]

    def blk(a):
        m = np.zeros((128, 2 * a.shape[1]), np.float32)
        m[0:64, 0 : a.shape[1]] = a
        m[64:128, a.shape[1] :] = a
        return m

    wk2 = blk(Wk).astype(bf)
    wqk2 = blk(Wqk).astype(bf)
    w2b = blk(W2).astype(bf)
    # w34: row 32*jq + 16*bh + h -> col 2*jq + bh = W3[h]
    w34 = np.zeros((128, 8), np.float32)
    for jq in range(4):
        for bh in range(2):
            w34[32 * jq + 16 * bh : 32 * jq + 16 * bh + 16, 2 * jq + bh] = W3[:, 0]
    w34 = w34.astype(bf)
    cols = np.zeros((128, 8), np.float32)
    cols[:, 0] = np.tile(inv1, 2)
    cols[:, 1] = np.tile(-mean1 * inv1, 2)
    cols[:, 2] = np.tile(1.0 - alpha1, 2)
    cols[:, 3] = np.tile(alpha1, 2)
    cols[:, 4] = np.tile(inv2, 8)
    cols[:, 5] = np.tile(-mean2 * inv2, 8)
    cols[:, 6] = np.tile(1.0 - alpha2, 8)
    cols[:, 7] = np.tile(alpha2, 8)
    onehot = np.zeros((2, 416), np.float32)
    onehot[0, 0:208] = 1.0
    onehot[1, 208:416] = 1.0
    return Wq, wk2, wqk2, w2b, w34, cols, onehot.astype(bf)


def kernel(query_emb, key_emb, seq_length, W1, alpha1, mean1, var1,
           W2, alpha2, mean2, var2, W3, _ngroups=NGROUPS_FULL):
    (Wq, wk2, wqk2, w2b, w34, cols, onehot) = _prep_consts(
        np.asarray(W1, np.float32), np.asarray(alpha1, np.float32),
        np.asarray(mean1, np.float32), np.asarray(var1, np.float32),
        np.asarray(W2, np.float32), np.asarray(alpha2, np.float32),
        np.asarray(mean2, np.float32), np.asarray(var2, np.float32),
        np.asarray(W3, np.float32))
    q = np.asarray(query_emb, np.float32)
    k = np.asarray(key_emb, np.float32)
    sl = np.asarray(seq_length).reshape(-1)

    if _ngroups not in _CACHE:
        _CACHE[_ngroups] = _build(_ngroups)
    nc = _CACHE[_ngroups]
    nb = 16 * _ngroups
    npair = nb // 2

    qW = q @ Wq  # [B, 64]

    in_maps = []
    for c in range(NCORES):
        b0 = c * BLOC
        qs = q[b0 : b0 + nb]
        qWs = qW[b0 : b0 + nb]
        sls = sl[b0 : b0 + nb]
        # pair layouts [128=(bhat,d), npair]
        qp_t = np.zeros((128, npair), np.float32)
        for bh in range(2):
            qp_t[64 * bh : 64 * bh + 64] = qs[bh::2].T
        # qw2 [2, 64*npair]: row k, cols 128*hq + 64*bh + h = qW[16g+2*(2m+k)+bh, h]
        # hq = 4g+m; pair j = 2*hq + k... j = 8g + 2m + k; batch row = 2j + bh
        qw2_t = np.zeros((2, 64 * npair), np.float32)
        for kk in range(2):
            # half-quad hq covers pairs (2*hq+kk); row b = 2*(2*hq+kk)+bh
            rows = qWs[4 * np.arange(npair // 2)[:, None, None] + 2 * kk
                       + np.array([0, 1])[None, :, None],
                       np.arange(64)[None, None, :]]      # [nhq, 2, 64]
            qw2_t[kk] = rows.reshape(-1)
        # mask [128, 32*ngroups]: cols 32g + (0:16 full | 16:32 partial), 2jj+bh
        t_full = np.arange(128)[:, None]
        t_part = np.arange(128)[:, None] + 128
        mk = np.zeros((128, 32 * _ngroups), np.float32)
        for g in range(_ngroups):
            slg = sls[16 * g : 16 * g + 16]  # rows 2jj+bh in order
            mk[:, 32 * g : 32 * g + 16] = t_full < slg[None, :]
            mp = (t_part < slg[None, :]).astype(np.float32)
            mp[80:] = 0.0
            mk[:, 32 * g + 16 : 32 * g + 32] = mp
        in_maps.append({
            "key": k[b0 : b0 + nb].reshape(nb * S, D),
            "qp": qp_t.astype(np.float32),
            "qw2": qw2_t.astype(bf),
            "onehot": onehot,
            "mask": mk.astype(bf),
            "wk2": wk2, "wqk2": wqk2, "w2b": w2b,
            "w34": w34, "cols": cols,
        })

    res = run_bass_kernel_spmd(nc, in_maps, list(range(NCORES)), trace=TRACE)
    global LAST_RESULT
    LAST_RESULT = res
    outs = []
    for c in range(NCORES):
        outs.append(res.results[c]["out"])
    return np.concatenate(outs, axis=0).astype(np.float32)
